# revision 3
# baseline (speedup 1.0000x reference)
"""CoLaKG model kernel for 8 Trainium2 NeuronCores (self-contained).

Pipeline (4 bass SPMD kernels; host does static prep + mechanical relayout only):
  K1 gemm : semantic projections (users+items) + merge, + s1/s2 GAT scalars
  K2 attn : item-neighbor GAT attention (dma_gather + softmax + weighted sum)
  K3 spmm : one LightGCN propagation layer (run 3x); dest-sharded PE segment-sum
  K4 final: gather 4 embedding tables at (user,item) rows, mean, dot product
"""
import copy
import numpy as np

import jax
jax.config.update("jax_compilation_cache_dir", "/tmp/.jax_bass_cache")
jax.config.update("jax_persistent_cache_min_entry_size_bytes", -1)
jax.config.update("jax_persistent_cache_min_compile_time_secs", 0.0)

import concourse.bass as bass
import concourse.mybir as mybir
from concourse.tile import TileContext
from concourse import bass_utils, library_config
import concourse.tile as tile_mod
from concourse.vector_clock import ScopedClock

F32 = mybir.dt.float32
I16 = mybir.dt.int16
AF = mybir.ActivationFunctionType

# ---------------------------------------------------------------- tile patch
MAX_WAITS = 1

def _split_sync_waits(nc, max_waits=MAX_WAITS):
    template = None
    counter = [0]
    for fn in nc.m.functions:
        for bb in fn.blocks:
            for inst in bb.instructions:
                if type(inst).__name__ == "InstNoOp":
                    template = copy.deepcopy(inst)
                    break
            if template is not None:
                break
        if template is not None:
            break
    for fn in nc.m.functions:
        for bb in fn.blocks:
            il = bb.instructions
            i = 0
            while i < len(il):
                inst = il[i]
                if template is None and type(inst).__name__ == "InstNoOp":
                    template = copy.deepcopy(inst)
                si = inst.sync_info
                if si is not None and si.on_wait is not None and len(si.on_wait) > max_waits:
                    assert template is not None, "no InstNoOp to clone"
                    waits = list(si.on_wait)
                    keep, rest = waits[:max_waits], waits[max_waits:]
                    si.on_wait.clear()
                    for w in keep:
                        si.on_wait.append(w)
                    carriers = []
                    while rest:
                        c = copy.deepcopy(template)
                        counter[0] += 1
                        c.name = f"I-waitsplit-{counter[0]}"
                        c.engine = inst.engine
                        c.sync_info = mybir.SyncInfo(on_wait=list(rest[:max_waits]), on_update=[])
                        carriers.append(c)
                        rest = rest[max_waits:]
                    for k, cinst in enumerate(carriers):
                        try:
                            nc.register_instruction(cinst, overwrite=True)
                        except Exception:
                            pass
                        il.insert(i + k, cinst)
                    i += len(carriers)
                i += 1

def _patched_drain_and_barrier(self, tick_clock, wait_clock):
    nc = self.nc
    nop0 = nc.sync.nop(nofuse=True, hint="predrain_waits")
    wait_clock.add_sem_waits(nop0.ins, ScopedClock({None: tick_clock.global_clock}))
    nc.sync.drain()
    nc.all_engine_barrier()
    assert self.sems is not None
    popped = nc._tile_sem_poison_stack.pop()
    assert popped is self._sem_poison
    nc.clear_and_free_semaphores(list(self.sems.allocated().values()))
    nc.all_engine_barrier()
    _split_sync_waits(nc)

tile_mod.TileContext._drain_and_barrier = _patched_drain_and_barrier

# ---------------------------------------------------------------- constants
NUM_USERS, NUM_ITEMS, D, SEM, HID, K = 60000, 30000, 64, 1024, 32, 32
N = NUM_USERS + NUM_ITEMS             # 90000
NPAD = 90112                          # 704*128
NCORE = 8
DPC = NPAD // NCORE                   # 11264 dest rows per core
DELTA = 4                             # dest cols per minirange
MR_PER_WIN = 96
WIN = DELTA * MR_PER_WIN              # 384 psum cols per window
NWIN = 30                             # 30*384 = 11520 >= 11264
SLOT = 128
UPC = NUM_USERS // NCORE              # 7500
IPC = NUM_ITEMS // NCORE              # 3750
IPAD = 3840
NBLK = IPAD // 128                    # 30
SUBB = 3                              # item blocks per attention sub-batch
NSUB = NBLK // SUBB                   # 10
B = 4096
BPC = B // NCORE                      # 512

_BUILT = {}


def _wrap_idx(idx_flat):
    """dma_gather idx layout: idx j -> [j%16, j//16], replicated to 8x16 partitions."""
    n = idx_flat.shape[0]
    assert n % 16 == 0
    blk = idx_flat.reshape(n // 16, 16).T.astype(np.int16)
    return np.tile(blk, (8, 1))


def _elu(nc, pool, out_ap, in_ap, shape, tag):
    """out = elu(in) = max(x,0) + exp(min(x,0)) - 1   (no Elu in ACT table)."""
    mn = pool.tile(shape, F32, tag=tag + "_mn")
    nc.vector.tensor_scalar_min(mn[:], in_ap, 0.0)
    ex = pool.tile(shape, F32, tag=tag + "_ex")
    nc.scalar.activation(ex[:], mn[:], AF.Exp, scale=1.0)
    mx = pool.tile(shape, F32, tag=tag + "_mx")
    nc.vector.tensor_scalar_max(mx[:], in_ap, 0.0)
    nc.vector.tensor_add(out_ap, mx[:], ex[:])
    nc.vector.tensor_scalar_add(out_ap, out_ap, -1.0)


# ================================================================ K1: GEMM
def _build_k1():
    nc = bass.Bass("TRN2", target_bir_lowering=False)
    xu = nc.dram_tensor("xu", [SEM, UPC], F32, kind="ExternalInput")
    xi = nc.dram_tensor("xi", [SEM, IPC], F32, kind="ExternalInput")
    wu = nc.dram_tensor("wu", [SEM, 64], F32, kind="ExternalInput")
    wi = nc.dram_tensor("wi", [SEM, 66], F32, kind="ExternalInput")
    bu = nc.dram_tensor("bu", [64, 1], F32, kind="ExternalInput")
    bi = nc.dram_tensor("bi", [66, 1], F32, kind="ExternalInput")
    eu = nc.dram_tensor("eu", [64, UPC], F32, kind="ExternalInput")
    ei = nc.dram_tensor("ei", [64, IPC], F32, kind="ExternalInput")
    ou = nc.dram_tensor("ou", [64, UPC], F32, kind="ExternalOutput")
    oi = nc.dram_tensor("oi", [64, IPC], F32, kind="ExternalOutput")
    os12 = nc.dram_tensor("os12", [2, IPC], F32, kind="ExternalOutput")

    with TileContext(nc) as tc:
        with tc.tile_pool(name="w", bufs=1) as wp, \
             tc.tile_pool(name="x", bufs=3) as xp, \
             tc.tile_pool(name="o", bufs=2) as op, \
             tc.tile_pool(name="ps", bufs=2, space="PSUM") as pp:
            wu_sb = wp.tile([128, SEM // 128, 64], F32, tag="wu")
            nc.sync.dma_start(wu_sb[:], wu[:].rearrange("(a p) m -> p a m", p=128))
            wi_sb = wp.tile([128, SEM // 128, 66], F32, tag="wi")
            nc.sync.dma_start(wi_sb[:], wi[:].rearrange("(a p) m -> p a m", p=128))
            bu_sb = wp.tile([64, 1], F32, tag="bu")
            nc.sync.dma_start(bu_sb[:], bu[:])
            bi_sb = wp.tile([66, 1], F32, tag="bi")
            nc.sync.dma_start(bi_sb[:], bi[:])

            def gemm(xten, eten, wtile, btile, oten, m, rows, R, s12=None):
                for t in range(rows // R):
                    xt = xp.tile([128, SEM // 128, R], F32, tag="xt")
                    nc.sync.dma_start(
                        xt[:], xten[:, t * R:(t + 1) * R].rearrange("(a p) r -> p a r", p=128))
                    ps = pp.tile([m, R], F32, tag="ps")
                    for kk in range(SEM // 128):
                        nc.tensor.matmul(ps[:], wtile[:, kk, :], xt[:, kk, :],
                                         start=(kk == 0), stop=(kk == SEM // 128 - 1))
                    xb = op.tile([64, R], F32, tag="xb")
                    nc.vector.tensor_scalar_add(xb[:], ps[0:64, :], btile[0:64, :])
                    mg = op.tile([64, R], F32, tag="mg")
                    _elu(nc, op, mg[:], xb[:], [64, R], "e1")
                    et = op.tile([64, R], F32, tag="et")
                    nc.sync.dma_start(et[:], eten[:, t * R:(t + 1) * R])
                    nc.vector.tensor_add(mg[:], mg[:], et[:])
                    nc.scalar.mul(mg[:], mg[:], 0.5)
                    nc.sync.dma_start(oten[:, t * R:(t + 1) * R], mg[:])
                    if s12 is not None:
                        sv = op.tile([2, R], F32, tag="sv")
                        nc.scalar.copy(sv[:], ps[64:66, :])
                        nc.sync.dma_start(s12[:, t * R:(t + 1) * R], sv[:])

            gemm(xu, eu, wu_sb, bu_sb, ou, 64, UPC, 500)
            gemm(xi, ei, wi_sb, bi_sb, oi, 66, IPC, 375, s12=os12)
    return nc


# ================================================================ K2: attention
def _build_k2():
    nc = bass.Bass("TRN2", target_bir_lowering=False)
    tbl = nc.dram_tensor("tbl", [NUM_ITEMS, 128], F32, kind="ExternalInput")
    adji = nc.dram_tensor("adji", [128, IPAD * K // 128], mybir.dt.int32, kind="ExternalInput")
    s2r = nc.dram_tensor("s2r", [128, NBLK * K], F32, kind="ExternalInput")
    itm = nc.dram_tensor("itm", [128, NBLK * 64], F32, kind="ExternalInput")
    oit = nc.dram_tensor("oit", [128, NBLK * 64], F32, kind="ExternalOutput")

    SLOTS = SUBB * 128 * K  # idxs per sub-batch
    with TileContext(nc) as tc:
        with tc.tile_pool(name="g", bufs=2) as gp, \
             tc.tile_pool(name="t", bufs=1) as tp, \
             tc.tile_pool(name="s", bufs=1) as sp, \
             tc.tile_pool(name="m", bufs=2) as mp:
            adj_sb = sp.tile([128, IPAD * K // 128], mybir.dt.int32, tag="adj")
            nc.sync.dma_start(adj_sb[:], adji[:])
            s2_sb = sp.tile([128, NBLK * K], F32, tag="s2")
            nc.sync.dma_start(s2_sb[:], s2r[:])
            itm_sb = sp.tile([128, NBLK * 64], F32, tag="itm")
            nc.sync.dma_start(itm_sb[:], itm[:])
            for u in range(NSUB):
                g = gp.tile([128, SUBB * K, 128], F32, tag="g")
                for jj in range(SLOTS // 128):
                    nc.gpsimd.indirect_dma_start(
                        out=g[:, jj, :], out_offset=None, in_=tbl[:],
                        in_offset=bass.IndirectOffsetOnAxis(
                            ap=adj_sb[:, u * (SLOTS // 128) + jj:u * (SLOTS // 128) + jj + 1],
                            axis=0))
                lg = mp.tile([128, SUBB * K], F32, tag="lg")
                nc.vector.tensor_add(lg[:], g[:, :, 64],
                                     s2_sb[:, u * SUBB * K:(u + 1) * SUBB * K])
                lr = mp.tile([128, SUBB * K], F32, tag="lr")
                nc.scalar.mul(lr[:], lg[:], 0.2)
                nc.vector.tensor_max(lg[:], lg[:], lr[:])
                ex = mp.tile([128, SUBB, K], F32, tag="ex")
                nc.scalar.activation(ex[:].rearrange("p a b -> p (a b)"), lg[:],
                                     AF.Exp, scale=1.0)
                sm = mp.tile([128, SUBB], F32, tag="sm")
                nc.vector.reduce_sum(sm[:], ex[:], axis=mybir.AxisListType.X)
                nc.vector.reciprocal(sm[:], sm[:])
                att = mp.tile([128, SUBB, K], F32, tag="att")
                for bb in range(SUBB):
                    nc.vector.tensor_scalar_mul(att[:, bb, :], ex[:, bb, :], sm[:, bb:bb + 1])
                tmp = tp.tile([128, SUBB, K, 64], F32, tag="tmp")
                av = att[:]
                att_b = bass.AP(av.tensor, av.offset, list(av.ap) + [[0, 64]])
                nc.vector.tensor_mul(
                    tmp[:], g[:].rearrange("p (b k) d -> p b k d", b=SUBB)[:, :, :, 0:64],
                    att_b)
                hp = mp.tile([128, SUBB, 64], F32, tag="hp")
                nc.vector.reduce_sum(hp[:], tmp[:].rearrange("p b k d -> p b d k"),
                                     axis=mybir.AxisListType.X)
                he = mp.tile([128, SUBB * 64], F32, tag="he")
                _elu(nc, mp, he[:], hp[:].rearrange("p b d -> p (b d)"),
                     [128, SUBB * 64], "e2")
                fo = mp.tile([128, SUBB * 64], F32, tag="fo")
                nc.vector.tensor_add(fo[:], he[:],
                                     itm_sb[:, u * SUBB * 64:(u + 1) * SUBB * 64])
                nc.scalar.mul(fo[:], fo[:], 0.5)
                nc.sync.dma_start(oit[:, u * SUBB * 64:(u + 1) * SUBB * 64], fo[:])
    return nc


# ================================================================ K3: spmm layer
def _build_k3():
    nc = bass.Bass("TRN2", target_bir_lowering=False)
    tbl = nc.dram_tensor("tbl", [NPAD, 64], F32, kind="ExternalInput")
    idx = nc.dram_tensor("idx", [128, NWIN * (MR_PER_WIN + 1)], mybir.dt.int32,
                         kind="ExternalInput")
    wreg = nc.dram_tensor("wreg", [128, NWIN * MR_PER_WIN * DELTA], F32,
                          kind="ExternalInput")
    wext = nc.dram_tensor("wext", [128, NWIN * WIN], F32, kind="ExternalInput")
    out = nc.dram_tensor("out", [64, NWIN * WIN], F32, kind="ExternalOutput")

    CH = MR_PER_WIN + 1
    with TileContext(nc) as tc:
        with tc.tile_pool(name="g", bufs=3) as gp, \
             tc.tile_pool(name="i", bufs=3) as ip, \
             tc.tile_pool(name="w", bufs=3) as wp, \
             tc.tile_pool(name="o", bufs=2) as op, \
             tc.tile_pool(name="ps", bufs=2, space="PSUM") as pp:
            for w in range(NWIN):
                it = ip.tile([128, CH], mybir.dt.int32, tag="idx")
                nc.sync.dma_start(it[:], idx[:, w * CH:(w + 1) * CH])
                gt = gp.tile([128, CH, 64], F32, tag="g")
                for j in range(CH):
                    nc.gpsimd.indirect_dma_start(
                        out=gt[:, j, :], out_offset=None, in_=tbl[:],
                        in_offset=bass.IndirectOffsetOnAxis(ap=it[:, j:j + 1], axis=0))
                wt = wp.tile([128, MR_PER_WIN * DELTA], F32, tag="w")
                nc.sync.dma_start(
                    wt[:], wreg[:, w * MR_PER_WIN * DELTA:(w + 1) * MR_PER_WIN * DELTA])
                we = wp.tile([128, WIN], F32, tag="we")
                nc.sync.dma_start(we[:], wext[:, w * WIN:(w + 1) * WIN])
                ps = pp.tile([64, WIN], F32, tag="ps")
                pse = pp.tile([64, WIN], F32, tag="pse")
                nc.tensor.matmul(pse[:], gt[:, MR_PER_WIN, :], we[:],
                                 start=True, stop=True)
                for j in range(MR_PER_WIN):
                    nc.tensor.matmul(
                        ps[:, j * DELTA:(j + 1) * DELTA],
                        gt[:, j, :], wt[:, j * DELTA:(j + 1) * DELTA],
                        start=True, stop=True)
                ot = op.tile([64, WIN], F32, tag="ot")
                nc.scalar.copy(ot[:], ps[:])
                nc.vector.tensor_add(ot[:], ot[:], pse[:])
                nc.sync.dma_start(out[:, w * WIN:(w + 1) * WIN], ot[:])
    return nc


# ================================================================ K4: final
def _build_k4():
    nc = bass.Bass("TRN2", target_bir_lowering=False)
    tbs = [nc.dram_tensor(f"tb{l}", [NPAD, 64], F32, kind="ExternalInput") for l in range(4)]
    fidx = nc.dram_tensor("fidx", [128, 8], mybir.dt.int32, kind="ExternalInput")
    out = nc.dram_tensor("out", [128, 4], F32, kind="ExternalOutput")

    with TileContext(nc) as tc:
        with tc.tile_pool(name="g", bufs=2) as gp, \
             tc.tile_pool(name="m", bufs=1) as mp:
            it = mp.tile([128, 8], mybir.dt.int32, tag="it")
            nc.sync.dma_start(it[:], fidx[:])
            acc = mp.tile([128, 8, 64], F32, tag="acc")
            for l in range(4):
                gt = gp.tile([128, 8, 64], F32, tag="g")
                for t in range(8):
                    nc.gpsimd.indirect_dma_start(
                        out=gt[:, t, :], out_offset=None, in_=tbs[l][:],
                        in_offset=bass.IndirectOffsetOnAxis(ap=it[:, t:t + 1], axis=0))
                if l == 0:
                    nc.vector.tensor_copy(acc[:], gt[:])
                else:
                    nc.vector.tensor_add(acc[:], acc[:], gt[:])
            nc.scalar.mul(acc[:], acc[:], 0.25)
            prod = mp.tile([128, 4, 64], F32, tag="prod")
            nc.vector.tensor_mul(prod[:], acc[:, 0:4, :], acc[:, 4:8, :])
            res = mp.tile([128, 4], F32, tag="res")
            nc.vector.reduce_sum(res[:], prod[:], axis=mybir.AxisListType.X)
            nc.sync.dma_start(out[:], res[:])
    return nc


# ================================================================ host packing
def _pack_spmm(rows, cols, vals):
    CH = MR_PER_WIN + 1
    idx_arr = np.zeros((NCORE, 128, NWIN * CH), dtype=np.int32)
    wreg = np.zeros((NCORE, 128, NWIN * MR_PER_WIN * DELTA), dtype=np.float32)
    wext = np.zeros((NCORE, 128, NWIN * WIN), dtype=np.float32)
    order = np.argsort(rows, kind="stable")
    rows_s, cols_s, vals_s = rows[order], cols[order], vals[order]
    core_of = rows_s // DPC
    for c in range(NCORE):
        cm = core_of == c
        r, co, v = rows_s[cm], cols_s[cm], vals_s[cm]
        base = c * DPC
        mini = (r - base) // DELTA
        extra_used = np.zeros(NWIN, np.int64)
        uniq, start_idx, cnt = np.unique(mini, return_index=True, return_counts=True)
        for mr, st, ct in zip(uniq, start_idx, cnt):
            w, j = mr // MR_PER_WIN, mr % MR_PER_WIN
            n = min(ct, SLOT)
            sl = np.arange(n)
            idx_arr[c, sl, w * CH + j] = co[st:st + n]
            dcol = (r[st:st + n] - base) - mr * DELTA
            wreg[c, sl, (w * MR_PER_WIN + j) * DELTA + dcol] = v[st:st + n]
            if ct > SLOT:
                ne = ct - SLOT
                u0 = extra_used[w]
                assert u0 + ne <= 128, f"extra chunk overflow win {w}: {u0}+{ne}"
                sle = np.arange(u0, u0 + ne)
                idx_arr[c, sle, w * CH + MR_PER_WIN] = co[st + SLOT:st + ct]
                dcole = (r[st + SLOT:st + ct] - base) - w * WIN
                wext[c, sle, w * WIN + dcole] = v[st + SLOT:st + ct]
                extra_used[w] = u0 + ne
    return idx_arr, wreg, wext


def _prep(inputs):
    p = {}
    users = np.asarray(inputs["users"]);   items = np.asarray(inputs["items"])
    adj = np.asarray(inputs["adj_matrix"])
    rows = np.asarray(inputs["graph_rows"]).astype(np.int64)
    cols = np.asarray(inputs["graph_cols"]).astype(np.int64)
    vals = np.asarray(inputs["graph_vals"]).astype(np.float32)
    W_att = np.asarray(inputs["W_att"]); a_att = np.asarray(inputs["a_att"])
    v1 = W_att @ a_att[:HID, 0]; v2 = W_att @ a_att[HID:, 0]

    p["xu"] = np.ascontiguousarray(np.asarray(inputs["user_semantic_emb"]).T)
    p["xi"] = np.ascontiguousarray(np.asarray(inputs["semantic_emb"]).T)
    p["wu"] = np.asarray(inputs["W_usem"])
    p["wi"] = np.concatenate([np.asarray(inputs["W_sem"]), v1[:, None], v2[:, None]],
                             axis=1).astype(np.float32)
    p["bu"] = np.asarray(inputs["b_usem"]).reshape(64, 1)
    p["bi"] = np.concatenate([np.asarray(inputs["b_sem"]), np.zeros(2, np.float32)]
                             ).reshape(66, 1).astype(np.float32)
    p["eu"] = np.ascontiguousarray(np.asarray(inputs["emb_user"]).T)
    p["ei"] = np.ascontiguousarray(np.asarray(inputs["emb_item"]).T)

    adj_pad = np.zeros((NCORE, IPAD, K), np.int64)
    for c in range(NCORE):
        adj_pad[c, :IPC] = adj[c * IPC:(c + 1) * IPC]
    # slot t = b*4096 + k*128 + pp -> gather out[pp, b*K+k] = adj[b*128+pp, k]
    slot_idx = np.transpose(adj_pad.reshape(NCORE, NBLK, 128, K), (0, 2, 1, 3))
    p["adji"] = np.ascontiguousarray(slot_idx.reshape(NCORE, 128, NBLK * K).astype(np.int32))

    p["spmm"] = _pack_spmm(rows, cols, vals)

    p["k4"] = []
    for c in range(NCORE):
        u = users[c * BPC:(c + 1) * BPC].astype(np.int64)
        it = items[c * BPC:(c + 1) * BPC].astype(np.int64) + NUM_USERS
        rws = np.concatenate([u, it])
        p["k4"].append(np.ascontiguousarray(rws.reshape(8, 128).T.astype(np.int32)))
    return p


import os as _os
import sys as _sys
import time as _time
_KTIME = bool(_os.environ.get("KTIME"))


def _tlog(msg):
    if _KTIME:
        print(f"[ktime {_time.time():.3f}] {msg}", file=_sys.stderr, flush=True)


def _run(name, builder, in_maps):
    if name not in _BUILT:
        _BUILT[name] = builder()
    t0 = _time.time()
    r = bass_utils.run_bass_kernel_spmd(
        _BUILT[name], in_maps, core_ids=list(range(NCORE))).results
    _tlog(f"dispatch {name}: {_time.time()-t0:.3f}s")
    return r


def kernel(**inputs):
    _tlog("kernel start")
    p = _prep(inputs)
    _tlog("prep done")

    # ---------------- K1
    maps = [{
        "xu": p["xu"][:, c * UPC:(c + 1) * UPC],
        "xi": p["xi"][:, c * IPC:(c + 1) * IPC],
        "wu": p["wu"], "wi": p["wi"], "bu": p["bu"], "bi": p["bi"],
        "eu": p["eu"][:, c * UPC:(c + 1) * UPC],
        "ei": p["ei"][:, c * IPC:(c + 1) * IPC],
    } for c in range(NCORE)]
    r1 = _run("k1", _build_k1, maps)
    users_m = np.concatenate([r1[c]["ou"].T for c in range(NCORE)], 0)
    items_m = np.concatenate([r1[c]["oi"].T for c in range(NCORE)], 0)
    s1 = np.concatenate([r1[c]["os12"][0] for c in range(NCORE)])
    s2 = np.concatenate([r1[c]["os12"][1] for c in range(NCORE)])

    # ---------------- K2
    tblA = np.ascontiguousarray(np.concatenate(
        [items_m, s1[:, None], np.zeros((NUM_ITEMS, 63), np.float32)], 1))
    maps = []
    for c in range(NCORE):
        s2c = np.zeros(IPAD, np.float32)
        s2c[:IPC] = s2[c * IPC:(c + 1) * IPC]
        s2r = np.transpose(np.broadcast_to(s2c.reshape(NBLK, 1, 128), (NBLK, K, 128)),
                           (2, 0, 1)).reshape(128, NBLK * K)
        imc = np.zeros((IPAD, 64), np.float32)
        imc[:IPC] = items_m[c * IPC:(c + 1) * IPC]
        itm = np.transpose(imc.reshape(NBLK, 128, 64), (1, 0, 2)).reshape(128, NBLK * 64)
        maps.append({"tbl": tblA, "adji": p["adji"][c],
                     "s2r": np.ascontiguousarray(s2r),
                     "itm": np.ascontiguousarray(itm)})
    r2 = _run("k2", _build_k2, maps)
    items_f = np.zeros((NUM_ITEMS, 64), np.float32)
    for c in range(NCORE):
        o = r2[c]["oit"].reshape(128, NBLK, 64).transpose(1, 0, 2).reshape(IPAD, 64)
        items_f[c * IPC:(c + 1) * IPC] = o[:IPC]

    # ---------------- K3 x3
    idx_arr, wreg, wext = p["spmm"]
    def run_layer(tbl_full):
        maps = [{"tbl": tbl_full, "idx": idx_arr[c], "wreg": wreg[c],
                 "wext": wext[c]} for c in range(NCORE)]
        r = _run("k3", _build_k3, maps)
        res = np.zeros((NPAD, 64), np.float32)
        for c in range(NCORE):
            res[c * DPC:(c + 1) * DPC] = r[c]["out"].T[:DPC]
        return res

    e0 = np.zeros((NPAD, 64), np.float32)
    e0[:NUM_USERS] = users_m
    e0[NUM_USERS:N] = items_f
    e1 = run_layer(e0)
    e2 = run_layer(e1)
    e3 = run_layer(e2)

    # ---------------- K4
    maps = [dict({f"tb{l}": tb for l, tb in enumerate((e0, e1, e2, e3))},
                 fidx=p["k4"][c]) for c in range(NCORE)]
    r4 = _run("k4", _build_k4, maps)
    gamma = np.zeros(B, np.float32)
    for c in range(NCORE):
        gamma[c * BPC:(c + 1) * BPC] = r4[c]["out"].T.reshape(BPC)
    return gamma

